# revision 7
# baseline (speedup 1.0000x reference)
"""MultiHeadCrossAttention Trainium2 Bass kernel (v2: deep-pipelined).

Sharding (8 cores): data-parallel over batch (2) x tensor-parallel over
head groups (4 groups of 4 heads).  Core c handles batch c//4, heads
4*(c%4) .. 4*(c%4)+3.  Each core computes a partial [Tq, D] output
(its heads' contribution through its Wo row-slice); the host sums the 4
partials per batch.

Device math per core (matmuls fp16 x fp16 -> fp32 PSUM):
  qT = Wq_s.T @ Xq.T          [256, Tq]   (head-dim on partitions)
  kT = Wk_s.T @ Xkv.T         [256, Tk]
  V  = Xkv @ Wv_s             [Tk, 256]   (+ ones column per head)
  St = kT_h.T @ qT_h          [128-tk-tile, 1024-q] scores^T, K=64
  E  = exp(St/8)              (ScalarE, scale folded into activation)
  P  = E * expb               expb = exp(bias^T) * mask^T  (host-built)
  [num^T; den] = [V_h|1].T @ P   [65, 512] psum accum over tk tiles
  stack = num^T * (1/den)     (recip on DVE, broadcast + muls on engines)
  partial = stack.T @ Wo_s    [Tq, D]  fp16 out; host sums partials

v2 structure: single linear emission, one (tqh x pair) "block" per
CH=1024 query columns; per tk-tile iteration emits scores -> exp ->
expb-mul -> attnV so every engine pipelines; q/k/v projections and the
out-projection are interleaved INTO block iterations (no serial phases);
X DMAs are split in halves so the first scores start ~18us in.  PSUM:
tag ps = 2x[128,1024] (4 banks), tag po = 4x[65,512] (4 banks).
"""

import os
from contextlib import ExitStack

import numpy as np

import concourse.bass as bass
import concourse.mybir as mybir
import concourse.tile as tile
from concourse import bacc
from concourse.bass_utils import run_bass_kernel_spmd

# Problem dims (hardcoded per contract).
D_MODEL = 1024
NUM_HEADS = 16
D_HEAD = 64
B = 2
TQ = 2048
TK = 2048
N_CORES = 8
HPC = 4  # heads per core
SCALE = 1.0 / 8.0  # 1/sqrt(D_HEAD)

F16 = mybir.dt.float16
F32 = mybir.dt.float32
NP_F16 = np.float16

NQ = 512   # matmul moving free-dim chunk (PSUM bank = 512 fp32)
CH = 1024  # scores tile width (2 PSUM banks)
NDT = D_MODEL // 128   # 8 contraction tiles for projections
PAIRS = HPC // 2       # 2
NTK = TK // 128        # 16
VW = D_HEAD + 1        # 65: V columns per head incl. ones column
NTQH = TQ // CH        # 2
NQC = CH // NQ         # 2


def build_nc():
    nc = bacc.Bacc("TRN2", target_bir_lowering=False, debug=False)

    xq_d = nc.dram_tensor("xqT", [D_MODEL, TQ], F16, kind="ExternalInput")
    xkv_d = nc.dram_tensor("xkvT", [D_MODEL, TK], F16, kind="ExternalInput")
    wq_d = nc.dram_tensor("wq", [D_MODEL, 256], F16, kind="ExternalInput")
    wk_d = nc.dram_tensor("wk", [D_MODEL, 256], F16, kind="ExternalInput")
    wv_d = nc.dram_tensor("wv", [D_MODEL, 256], F16, kind="ExternalInput")
    wo_d = nc.dram_tensor("wo", [256, D_MODEL], F16, kind="ExternalInput")
    eb_d = nc.dram_tensor("expb", [HPC, TK, TQ], F16, kind="ExternalInput")
    out_d = nc.dram_tensor("out", [TQ, D_MODEL], F16, kind="ExternalOutput")

    with ExitStack() as ctx:
        tc = ctx.enter_context(tile.TileContext(nc))
        wpool = ctx.enter_context(tc.tile_pool(name="wpool", bufs=1))
        qkpool = ctx.enter_context(tc.tile_pool(name="qkpool", bufs=1))
        xpool = ctx.enter_context(tc.tile_pool(name="xpool", bufs=1))
        ebpool = ctx.enter_context(tc.tile_pool(name="ebpool", bufs=8))
        ppool = ctx.enter_context(tc.tile_pool(name="ppool", bufs=6))
        opool = ctx.enter_context(tc.tile_pool(name="opool", bufs=3))
        npool = ctx.enter_context(tc.tile_pool(name="npool", bufs=2))
        psum = ctx.enter_context(tc.tile_pool(name="psum", bufs=2, space="PSUM"))

        wq_sb = wpool.tile([128, NDT, 256], F16, tag="wq")
        wk_sb = wpool.tile([128, NDT, 256], F16, tag="wk")
        wv_sb = wpool.tile([128, NDT, 256], F16, tag="wv")
        wo_sb = wpool.tile([128, PAIRS, D_MODEL], F16, tag="wo")
        qT_sb = qkpool.tile([128, PAIRS, TQ], F16, tag="qT")
        kT_sb = qkpool.tile([128, PAIRS, TK], F16, tag="kT")
        v_sb = qkpool.tile([128, NTK, HPC * VW], F16, tag="v")
        stack_sb = qkpool.tile([128, PAIRS, TQ], F16, tag="stack")
        xkv_sb = [xpool.tile([128, TK], F16, tag=f"xkv{dt}", name="xkv_sb") for dt in range(NDT)]
        xq_sb = [xpool.tile([128, TQ], F16, tag=f"xq{dt}", name="xq_sb") for dt in range(NDT)]

        nc.gpsimd.memset(v_sb[:], 1.0)

        # ---- DMA emission order = sync queue order (halved X transfers so
        # the first scores tile can launch after ~5.5MB instead of ~10MB)
        nc.sync.dma_start(out=wk_sb[:], in_=wk_d.ap().rearrange("(t p) j -> p t j", p=128))
        for dt in range(NDT):
            nc.sync.dma_start(out=xkv_sb[dt][:, 0:CH], in_=xkv_d[dt * 128 : (dt + 1) * 128, 0:CH])
        nc.sync.dma_start(out=wv_sb[:], in_=wv_d.ap().rearrange("(t p) j -> p t j", p=128))
        nc.sync.dma_start(out=wq_sb[:], in_=wq_d.ap().rearrange("(t p) j -> p t j", p=128))
        for dt in range(NDT):
            nc.sync.dma_start(out=xq_sb[dt][:, 0:CH], in_=xq_d[dt * 128 : (dt + 1) * 128, 0:CH])

        # eb prefetch ring: global index g = (tqh*2+pair)*16 + t
        ebs = {}

        def ensure_eb(g):
            if g in ebs or not (0 <= g < 4 * NTK):
                return
            tqh, rem = divmod(g, 2 * NTK)
            pair, t = divmod(rem, NTK)
            c0 = tqh * CH
            eb_t = ebpool.tile([128, 2, CH], F16, tag="eb", name="eb_t")
            nc.sync.dma_start(
                out=eb_t[:],
                in_=eb_d.ap()[2 * pair : 2 * pair + 2, t * 128 : (t + 1) * 128,
                              c0 : c0 + CH].rearrange("h p q -> p h q"),
            )
            ebs[g] = eb_t

        for g in range(4):
            ensure_eb(g)
        for dt in range(NDT):
            nc.sync.dma_start(out=xkv_sb[dt][:, CH:TK], in_=xkv_d[dt * 128 : (dt + 1) * 128, CH:TK])
        for dt in range(NDT):
            nc.sync.dma_start(out=xq_sb[dt][:, CH:TQ], in_=xq_d[dt * 128 : (dt + 1) * 128, CH:TQ])
        nc.sync.dma_start(out=wo_sb[:], in_=wo_d.ap().rearrange("(t p) m -> p t m", p=128))

        # ---- emission helpers
        def proj_chunk(which, pair, c0):
            wsb, xsb, dst = {
                "k": (wk_sb, xkv_sb, kT_sb),
                "q": (wq_sb, xq_sb, qT_sb),
            }[which]
            ps = psum.tile([128, CH], F32, tag="ps", name="ps")
            for dt in range(NDT):
                for q0 in range(0, CH, NQ):
                    nc.tensor.matmul(
                        ps[:, q0 : q0 + NQ],
                        wsb[:, dt, pair * 128 : (pair + 1) * 128],
                        xsb[dt][:, c0 + q0 : c0 + q0 + NQ],
                        start=(dt == 0),
                        stop=(dt == NDT - 1),
                    )
            nc.vector.tensor_copy(dst[:, pair, c0 : c0 + CH], ps[:])

        def v_tile(t):
            ps = psum.tile([128, 256], F32, tag="ps", name="psv")
            for dt in range(NDT):
                nc.tensor.matmul(
                    ps[:],
                    xkv_sb[dt][:, t * 128 : (t + 1) * 128],
                    wv_sb[:, dt, :],
                    start=(dt == 0),
                    stop=(dt == NDT - 1),
                )
            nc.vector.tensor_copy(
                v_sb[:, t, :].rearrange("p (h w) -> p h w", w=VW)[:, :, 0:D_HEAD],
                ps[:].rearrange("p (h w) -> p h w", w=D_HEAD),
            )

        orow = [0]

        def outproj_row(r):
            pf = psum.tile([128, CH], F32, tag="ps", name="pf")
            for pair in range(PAIRS):
                for m0 in range(0, D_MODEL, NQ):
                    nc.tensor.matmul(
                        pf[:, m0 : m0 + NQ],
                        stack_sb[:, pair, r * 128 : (r + 1) * 128],
                        wo_sb[:, pair, m0 : m0 + NQ],
                        start=(pair == 0),
                        stop=(pair == PAIRS - 1),
                    )
            osb = opool.tile([128, D_MODEL], F16, tag="osb", name="osb")
            # gpsimd can't read PSUM; drain rows (ACT idle by then) use scalar
            eng = nc.vector.tensor_copy if r < 8 else nc.scalar.copy
            eng(osb[:], pf[:])
            nc.sync.dma_start(out=out_d[r * 128 : (r + 1) * 128, :], in_=osb[:])

        def block(tqh, pair, interleave):
            """One CH-wide attention block: all 16 tk tiles for 2 heads."""
            c0 = tqh * CH
            po = {}
            for t in range(NTK):
                g = (tqh * 2 + pair) * NTK + t
                ensure_eb(g + 4)
                eb_t = ebs.pop(g)
                pss = []
                for hh in range(2):
                    ps = psum.tile([128, CH], F32, tag="ps", name="psc")
                    for q0 in range(0, CH, NQ):
                        nc.tensor.matmul(
                            ps[:, q0 : q0 + NQ],
                            kT_sb[hh * 64 : (hh + 1) * 64, pair, t * 128 : (t + 1) * 128],
                            qT_sb[hh * 64 : (hh + 1) * 64, pair, c0 + q0 : c0 + q0 + NQ],
                            start=True,
                            stop=True,
                        )
                    pss.append(ps)
                for fn in interleave.get(t, ()):
                    fn()
                p_t = ppool.tile([128, 2, CH], F16, tag="p", name="p_t")
                for hh in range(2):
                    nc.scalar.activation(
                        out=p_t[:, hh, :], in_=pss[hh][:],
                        func=mybir.ActivationFunctionType.Exp, scale=SCALE,
                    )
                for hh in range(2):
                    # offload 1/4 of the expb-muls to gpsimd (SBUF-only op)
                    eng = nc.gpsimd if (t % 2 == 1 and hh == 1) else nc.vector
                    eng.tensor_mul(p_t[:, hh, :], p_t[:, hh, :], eb_t[:, hh, :])
                for hh in range(2):
                    h = 2 * pair + hh
                    for qi in range(NQC):
                        if t == 0:
                            po[hh, qi] = psum.tile([VW, NQ], F32, tag="po", bufs=4, name="po")
                        nc.tensor.matmul(
                            po[hh, qi][:],
                            v_sb[:, t, h * VW : (h + 1) * VW],
                            p_t[:, hh, qi * NQ : (qi + 1) * NQ],
                            start=(t == 0),
                            stop=(t == NTK - 1),
                        )
            # normalize: gather num|den to SBUF (gpsimd), batched reciprocal
            u65 = {}
            sums_t = npool.tile([4, NQ], F16, tag="sums", name="sums_t")
            for r, (hh, qi) in enumerate([(0, 0), (0, 1), (1, 0), (1, 1)]):
                u = npool.tile([VW, NQ], F16, tag="u65", bufs=4, name="u65_t")
                nc.vector.tensor_copy(u[:], po[hh, qi][:])
                nc.sync.dma_start(out=sums_t[r : r + 1, :], in_=u[64:65, :])
                u65[hh, qi] = u
            recip_f = npool.tile([4, NQ], F32, tag="recf", name="recip_f")
            nc.vector.reciprocal(out=recip_f[:], in_=sums_t[:])
            recip_h = npool.tile([4, NQ], F16, tag="rech", name="recip_h")
            nc.vector.tensor_copy(recip_h[:], recip_f[:])
            for r, (hh, qi) in enumerate([(0, 0), (0, 1), (1, 0), (1, 1)]):
                # broadcast needs its input on partition 0: DMA the row down
                r_t = npool.tile([1, NQ], F16, tag="r", bufs=4, name="r_t")
                nc.sync.dma_start(out=r_t[:], in_=recip_h[r : r + 1, :])
                rb = npool.tile([64, NQ], F16, tag="rb", bufs=4, name="rb_t")
                nc.gpsimd.partition_broadcast(rb[:], r_t[:])
                qg = tqh * NQC + qi
                nc.vector.tensor_mul(
                    stack_sb[hh * 64 : (hh + 1) * 64, pair, qg * NQ : (qg + 1) * NQ],
                    u65[hh, qi][0:64, :],
                    rb[:],
                )

        # ---- phase A minimum: kT/qT (pair 0, chunk 0) so scores can start
        proj_chunk("k", 0, 0)
        proj_chunk("q", 0, 0)

        # ---- blocks with interleaved projections / out-projections
        il00 = {
            0: [lambda: v_tile(0), lambda: v_tile(1)],
            1: [lambda: v_tile(2), lambda: v_tile(3)],
            2: [lambda: v_tile(4), lambda: v_tile(5)],
            3: [lambda: v_tile(6), lambda: v_tile(7)],
            4: [lambda: proj_chunk("k", 0, CH)],
            5: [lambda: proj_chunk("k", 1, 0)],
            6: [lambda: v_tile(8), lambda: v_tile(9)],
            7: [lambda: v_tile(10), lambda: v_tile(11)],
            8: [lambda: v_tile(12), lambda: v_tile(13)],
            9: [lambda: v_tile(14), lambda: v_tile(15)],
            10: [lambda: proj_chunk("q", 1, 0)],
            11: [lambda: proj_chunk("k", 1, CH)],
            12: [lambda: proj_chunk("q", 0, CH)],
            13: [lambda: proj_chunk("q", 1, CH)],
        }
        block(0, 0, il00)
        block(0, 1, {})
        il10 = {2 * i + 1: [lambda r=i: outproj_row(r)] for i in range(8)}
        block(1, 0, il10)
        block(1, 1, {})
        for r in range(8, 16):
            outproj_row(r)

    nc.compile()
    return nc


_NC = None
LAST_RESULTS = None


def _get_nc():
    global _NC
    if _NC is None:
        _NC = build_nc()
    return _NC


def _shard_inputs(query, key_value, mask, rel_pos_bias, Wq, Wkv, Wo):
    """Build the 8 per-core input maps (host-side transposes + exp-bias)."""
    in_maps = []
    w_f16 = {
        "Wq": Wq.astype(NP_F16),
        "Wo": Wo.astype(NP_F16),
        "Wkv": Wkv.astype(NP_F16),
    }
    for c in range(N_CORES):
        b = c // (N_CORES // B)
        g = c % (N_CORES // B)
        cs = slice(g * HPC * D_HEAD, (g + 1) * HPC * D_HEAD)
        hs = slice(g * HPC, (g + 1) * HPC)
        # expb = exp(bias)^T * mask^T   (fp32 exp, fp16 ship)
        eb = np.exp(rel_pos_bias[hs].astype(np.float32)).transpose(0, 2, 1)
        eb = eb * mask[b, 0].T[None].astype(np.float32)
        in_maps.append({
            "xqT": np.ascontiguousarray(query[b].T).astype(NP_F16),
            "xkvT": np.ascontiguousarray(key_value[b].T).astype(NP_F16),
            "wq": w_f16["Wq"][:, cs].copy(),
            "wk": w_f16["Wkv"][:, cs].copy(),
            "wv": w_f16["Wkv"][:, D_MODEL + cs.start : D_MODEL + cs.stop].copy(),
            "wo": w_f16["Wo"][cs, :].copy(),
            "expb": eb.astype(NP_F16),
        })
    return in_maps


def kernel(query, key_value, mask, rel_pos_bias, Wq, Wkv, Wo):
    global LAST_RESULTS
    query, key_value, mask, rel_pos_bias, Wq, Wkv, Wo = (
        np.asarray(a) for a in (query, key_value, mask, rel_pos_bias, Wq, Wkv, Wo)
    )
    nc = _get_nc()
    in_maps = _shard_inputs(query, key_value, mask, rel_pos_bias, Wq, Wkv, Wo)
    res = run_bass_kernel_spmd(nc, in_maps, core_ids=list(range(N_CORES)))
    LAST_RESULTS = res
    gpc = N_CORES // B  # cores per batch group
    out = np.stack([
        sum(res.results[b * gpc + i]["out"].astype(np.float32) for i in range(gpc))
        for b in range(B)
    ])
    return out


# revision 13
# speedup vs baseline: 1.0064x; 1.0064x over previous
"""MultiHeadCrossAttention Trainium2 Bass kernel (v3: decoupled rings).

Sharding (8 cores): data-parallel over batch (2) x tensor-parallel over
head groups (4 groups of 4 heads).  Core c handles batch c//4, heads
4*(c%4) .. 4*(c%4)+3.  Each core computes a partial [Tq, D] output; the
host sums the 4 partials per batch.

Device math per core (matmuls fp16 x fp16 -> fp32 PSUM):
  qT = Wq_s.T @ Xq.T          [256, Tq]
  kT = Wk_s.T @ Xkv.T         [256, Tk]
  V  = Xkv @ Wv_s             [Tk, 256]   (+ ones column per head)
  St = kT_h.T @ qT_h          [128, 1024] scores^T per tk tile, K=64
  E  = exp(St/8)              (ScalarE)
  P  = E * expb               (DVE, 1/4 offloaded to gpsimd)
  [num^T; den] = [V_h|1].T @ P   4 psum accumulators (hh x qi)
  stack = num^T * (1/den)     (approx reciprocal + broadcast muls)
  partial = stack.T @ Wo_s    fp16 out; host sums partials

Pipelining: the scalar engine's exp (128 x [128,1024] tiles ~ 145us) is
the pace target.  PSUM tags: "ps" 2x[128,1024] dedicated to scores (the
exp pipeline never waits on anything but exp itself), "aux" 1x[128,1024]
for interleaved projection / V / out-projection tiles, "po" 4x[65,512]
attnV accumulators (both heads in parallel, no end-of-block sweep).
X DMAs are halved so the first scores tile launches ~19us in; V tiles
and remaining projection chunks stream through "aux" inside the first
blocks; out-projection rows ride inside later blocks; the drain reuses
the idle "ps" ring + scalar engine.
"""

import os
from contextlib import ExitStack

import numpy as np

import concourse.bass as bass
import concourse.mybir as mybir
import concourse.tile as tile
from concourse import bacc
from concourse.bass_utils import run_bass_kernel_spmd

# Problem dims (hardcoded per contract).
D_MODEL = 1024
NUM_HEADS = 16
D_HEAD = 64
B = 2
TQ = 2048
TK = 2048
N_CORES = 8
HPC = 4  # heads per core
SCALE = 1.0 / 8.0  # 1/sqrt(D_HEAD)

F16 = mybir.dt.float16
F32 = mybir.dt.float32
NP_F16 = np.float16

NQ = 512
CH = 1024
NDT = D_MODEL // 128   # 8
PAIRS = HPC // 2       # 2
NTK = TK // 128        # 16
VW = D_HEAD + 1        # 65
NTQH = TQ // CH        # 2
NQC = CH // NQ         # 2


def build_nc():
    nc = bacc.Bacc("TRN2", target_bir_lowering=False, debug=False)

    xq_d = nc.dram_tensor("xqT", [D_MODEL, TQ], F16, kind="ExternalInput")
    xkv_d = nc.dram_tensor("xkvT", [D_MODEL, TK], F16, kind="ExternalInput")
    wq_d = nc.dram_tensor("wq", [D_MODEL, 256], F16, kind="ExternalInput")
    wk_d = nc.dram_tensor("wk", [D_MODEL, 256], F16, kind="ExternalInput")
    wv_d = nc.dram_tensor("wv", [D_MODEL, 256], F16, kind="ExternalInput")
    wo_d = nc.dram_tensor("wo", [256, D_MODEL], F16, kind="ExternalInput")
    eb_d = nc.dram_tensor("expb", [HPC, TK, TQ], F16, kind="ExternalInput")
    out_d = nc.dram_tensor("out", [TQ, D_MODEL], F16, kind="ExternalOutput")

    with ExitStack() as ctx:
        tc = ctx.enter_context(tile.TileContext(nc))
        wpool = ctx.enter_context(tc.tile_pool(name="wpool", bufs=1))
        qkpool = ctx.enter_context(tc.tile_pool(name="qkpool", bufs=1))
        xpool = ctx.enter_context(tc.tile_pool(name="xpool", bufs=1))
        ebpool = ctx.enter_context(tc.tile_pool(name="ebpool", bufs=6))
        papool = ctx.enter_context(tc.tile_pool(name="papool", bufs=4))
        pbpool = ctx.enter_context(tc.tile_pool(name="pbpool", bufs=17))
        opool = ctx.enter_context(tc.tile_pool(name="opool", bufs=3))
        npool = ctx.enter_context(tc.tile_pool(name="npool", bufs=2))
        psum = ctx.enter_context(tc.tile_pool(name="psum", bufs=2, space="PSUM"))

        wq_sb = wpool.tile([128, NDT, 256], F16, tag="wq")
        wk_sb = wpool.tile([128, NDT, 256], F16, tag="wk")
        wv_sb = wpool.tile([128, NDT, 256], F16, tag="wv")
        wo_sb = wpool.tile([128, PAIRS, D_MODEL], F16, tag="wo")
        qT_sb = qkpool.tile([128, PAIRS, TQ], F16, tag="qT")
        kT_sb = qkpool.tile([128, PAIRS, TK], F16, tag="kT")
        v_sb = qkpool.tile([128, NTK, HPC * VW], F16, tag="v")
        stack_sb = qkpool.tile([128, PAIRS, TQ], F16, tag="stack")
        xkv_sb = [xpool.tile([128, TK], F16, tag=f"xkv{dt}", name="xkv_sb") for dt in range(NDT)]
        xq_sb = [xpool.tile([128, TQ], F16, tag=f"xq{dt}", name="xq_sb") for dt in range(NDT)]

        nc.gpsimd.memset(v_sb[:], 1.0)

        # ---- DMA emission order = sync queue order
        nc.sync.dma_start(out=wk_sb[:], in_=wk_d.ap().rearrange("(t p) j -> p t j", p=128))
        for dt in range(NDT):
            nc.sync.dma_start(out=xkv_sb[dt][:, 0:CH], in_=xkv_d[dt * 128 : (dt + 1) * 128, 0:CH])
        nc.sync.dma_start(out=wv_sb[:], in_=wv_d.ap().rearrange("(t p) j -> p t j", p=128))
        nc.sync.dma_start(out=wq_sb[:], in_=wq_d.ap().rearrange("(t p) j -> p t j", p=128))
        for dt in range(NDT):
            nc.sync.dma_start(out=xq_sb[dt][:, 0:CH], in_=xq_d[dt * 128 : (dt + 1) * 128, 0:CH])

        ebs = {}

        def ensure_eb(g):
            if g in ebs or not (0 <= g < 4 * NTK):
                return
            tqh, rem = divmod(g, 2 * NTK)
            pair, t = divmod(rem, NTK)
            c0 = tqh * CH
            eb_t = ebpool.tile([128, 2, CH], F16, tag="eb", name="eb_t")
            nc.sync.dma_start(
                out=eb_t[:],
                in_=eb_d.ap()[2 * pair : 2 * pair + 2, t * 128 : (t + 1) * 128,
                              c0 : c0 + CH].rearrange("h p q -> p h q"),
            )
            ebs[g] = eb_t

        for g in range(4):
            ensure_eb(g)
        for dt in range(NDT):
            nc.sync.dma_start(out=xkv_sb[dt][:, CH:TK], in_=xkv_d[dt * 128 : (dt + 1) * 128, CH:TK])
        for dt in range(NDT):
            nc.sync.dma_start(out=xq_sb[dt][:, CH:TQ], in_=xq_d[dt * 128 : (dt + 1) * 128, CH:TQ])
        nc.sync.dma_start(out=wo_sb[:], in_=wo_d.ap().rearrange("(t p) m -> p t m", p=128))

        # ---- emission helpers (interleaved work uses the "aux" psum tag so
        # the scores ring never waits on cross-engine releases)
        def proj_chunk(which, pair, c0):
            wsb, xsb, dst = {
                "k": (wk_sb, xkv_sb, kT_sb),
                "q": (wq_sb, xq_sb, qT_sb),
            }[which]
            ps = psum.tile([128, CH], F32, tag="aux", bufs=1, name="ps")
            for dt in range(NDT):
                for q0 in range(0, CH, NQ):
                    nc.tensor.matmul(
                        ps[:, q0 : q0 + NQ],
                        wsb[:, dt, pair * 128 : (pair + 1) * 128],
                        xsb[dt][:, c0 + q0 : c0 + q0 + NQ],
                        start=(dt == 0),
                        stop=(dt == NDT - 1),
                    )
            nc.vector.tensor_copy(dst[:, pair, c0 : c0 + CH], ps[:])

        def v_tile(t):
            ps = psum.tile([128, 256], F32, tag="aux", bufs=1, name="psv")
            for dt in range(NDT):
                nc.tensor.matmul(
                    ps[:],
                    xkv_sb[dt][:, t * 128 : (t + 1) * 128],
                    wv_sb[:, dt, :],
                    start=(dt == 0),
                    stop=(dt == NDT - 1),
                )
            nc.vector.tensor_copy(
                v_sb[:, t, :].rearrange("p (h w) -> p h w", w=VW)[:, :, 0:D_HEAD],
                ps[:].rearrange("p (h w) -> p h w", w=D_HEAD),
            )

        def outproj_row(r, drain=False):
            # drain rows reuse the idle scores ring + scalar engine
            pf = psum.tile([128, CH], F32, tag="ps" if drain else "aux",
                           bufs=2 if drain else 1, name="pf")
            for pair in range(PAIRS):
                for m0 in range(0, D_MODEL, NQ):
                    nc.tensor.matmul(
                        pf[:, m0 : m0 + NQ],
                        stack_sb[:, pair, r * 128 : (r + 1) * 128],
                        wo_sb[:, pair, m0 : m0 + NQ],
                        start=(pair == 0),
                        stop=(pair == PAIRS - 1),
                    )
            osb = opool.tile([128, D_MODEL], F16, tag="osb", name="osb")
            (nc.scalar.copy if drain else nc.vector.tensor_copy)(osb[:], pf[:])
            nc.sync.dma_start(out=out_d[r * 128 : (r + 1) * 128, :], in_=osb[:])

        def block(tqh, pair, interleave):
            """One CH-wide attention block: 16 tk tiles x 2 heads.

            Head hh0 accumulates attnV inline per t (po ring 2 slots);
            head hh1's P tiles are retained (pbpool) and swept at block
            end into the same 2 slots after hh0 spills to SBUF."""
            c0 = tqh * CH
            po = {}
            pB = []
            for t in range(NTK):
                g = (tqh * 2 + pair) * NTK + t
                ensure_eb(g)
                ensure_eb(g + 6)
                eb_t = ebs.pop(g)
                pss = []
                for hh in range(2):
                    ps = psum.tile([128, CH], F32, tag="ps", name="psc")
                    for q0 in range(0, CH, NQ):
                        nc.tensor.matmul(
                            ps[:, q0 : q0 + NQ],
                            kT_sb[hh * 64 : (hh + 1) * 64, pair, t * 128 : (t + 1) * 128],
                            qT_sb[hh * 64 : (hh + 1) * 64, pair, c0 + q0 : c0 + q0 + NQ],
                            start=True,
                            stop=True,
                        )
                    pss.append(ps)
                for fn in interleave.get(t, ()):
                    fn()
                pa = papool.tile([128, CH], F16, tag="pa", name="pa_t")
                pb = pbpool.tile([128, CH], F16, tag="pb", name="pb_t")
                pB.append(pb)
                for hh, p_t in ((0, pa), (1, pb)):
                    nc.scalar.activation(
                        out=p_t[:], in_=pss[hh][:],
                        func=mybir.ActivationFunctionType.Exp, scale=SCALE,
                    )
                for hh, p_t in ((0, pa), (1, pb)):
                    eng = nc.gpsimd if (t % 2 == 1 and hh == 1) else nc.vector
                    eng.tensor_mul(p_t[:], p_t[:], eb_t[:, hh, :])
                h = 2 * pair
                for qi in range(NQC):
                    if t == 0:
                        po[qi] = psum.tile([VW, NQ], F32, tag="po", bufs=2, name="po")
                    nc.tensor.matmul(
                        po[qi][:],
                        v_sb[:, t, h * VW : (h + 1) * VW],
                        pa[:, qi * NQ : (qi + 1) * NQ],
                        start=(t == 0),
                        stop=(t == NTK - 1),
                    )
            # spill hh0, freeing the po slots for the hh1 sweep
            order = [(0, 0), (0, 1), (1, 0), (1, 1)]
            u65 = {}
            sums_t = npool.tile([4, NQ], F16, tag="sums", name="sums_t")

            def spill(hh, qi, r, ptile):
                u = npool.tile([VW, NQ], F16, tag="u65", bufs=4, name="u65_t")
                nc.vector.tensor_copy(u[:], ptile[:])
                nc.sync.dma_start(out=sums_t[r : r + 1, :], in_=u[64:65, :])
                u65[hh, qi] = u

            spill(0, 0, 0, po[0])
            spill(0, 1, 1, po[1])
            # hh1 sweep (scores of the next block proceed concurrently on PE)
            h = 2 * pair + 1
            po1 = {}
            for t in range(NTK):
                for qi in range(NQC):
                    if t == 0:
                        po1[qi] = psum.tile([VW, NQ], F32, tag="po", bufs=2, name="po")
                    nc.tensor.matmul(
                        po1[qi][:],
                        v_sb[:, t, h * VW : (h + 1) * VW],
                        pB[t][:, qi * NQ : (qi + 1) * NQ],
                        start=(t == 0),
                        stop=(t == NTK - 1),
                    )
            spill(1, 0, 2, po1[0])
            spill(1, 1, 3, po1[1])
            # batched approx reciprocal + broadcast muls into stack
            sums32 = npool.tile([4, NQ], F32, tag="sums32", bufs=1, name="sums32")
            nc.vector.tensor_copy(sums32[:], sums_t[:])
            rec32 = npool.tile([4, NQ], F32, tag="rec32", bufs=1, name="rec32")
            nc.vector.reciprocal_approx_fast(out=rec32[:], in_=sums32[:])
            rech = npool.tile([4, NQ], F16, tag="rech", bufs=1, name="rech")
            nc.vector.tensor_copy(rech[:], rec32[:])
            for r, (hh, qi) in enumerate(order):
                if r == 0:
                    src = rech[0:1, :]
                else:
                    r_t = npool.tile([1, NQ], F16, tag="r", bufs=2, name="r_t")
                    nc.sync.dma_start(out=r_t[:], in_=rech[r : r + 1, :])
                    src = r_t[:]
                rb = npool.tile([64, NQ], F16, tag="rb", bufs=2, name="rb_t")
                nc.gpsimd.partition_broadcast(rb[:], src)
                qg = tqh * NQC + qi
                nc.vector.tensor_mul(
                    stack_sb[hh * 64 : (hh + 1) * 64, pair, qg * NQ : (qg + 1) * NQ],
                    u65[hh, qi][0:64, :],
                    rb[:],
                )

        # ---- phase A minimum: enough projections for block (0,0) + (0,1)
        proj_chunk("k", 0, 0)
        proj_chunk("k", 1, 0)
        proj_chunk("q", 0, 0)
        v_tile(0)
        v_tile(1)

        # ---- blocks with interleaved work (aux ring: 1 item per iteration)
        il00 = {}
        for i in range(6):                    # V2..V7 at iters 0..5
            il00[i] = [lambda t=i + 2: v_tile(t)]
        il00[6] = [lambda: proj_chunk("k", 0, CH)]
        for i in range(7, 15):                # V8..V15 at iters 7..14
            il00[i] = [lambda t=i + 1: v_tile(t)]
        il00[15] = [lambda: proj_chunk("q", 1, 0)]
        block(0, 0, il00)

        il01 = {
            0: [lambda: proj_chunk("q", 0, CH)],
            2: [lambda: proj_chunk("k", 1, CH)],
            4: [lambda: proj_chunk("q", 1, CH)],
        }
        block(0, 1, il01)

        il10 = {2 * i + 1: [lambda r=i: outproj_row(r)] for i in range(8)}
        block(1, 0, il10)
        block(1, 1, {})
        for r in range(8, 16):
            outproj_row(r, drain=True)

    nc.compile()
    return nc


_NC = None
LAST_RESULTS = None


def _get_nc():
    global _NC
    if _NC is None:
        _NC = build_nc()
    return _NC


def _shard_inputs(query, key_value, mask, rel_pos_bias, Wq, Wkv, Wo):
    """Build the 8 per-core input maps (host-side transposes + exp-bias)."""
    in_maps = []
    w_f16 = {
        "Wq": Wq.astype(NP_F16),
        "Wo": Wo.astype(NP_F16),
        "Wkv": Wkv.astype(NP_F16),
    }
    for c in range(N_CORES):
        b = c // (N_CORES // B)
        g = c % (N_CORES // B)
        cs = slice(g * HPC * D_HEAD, (g + 1) * HPC * D_HEAD)
        hs = slice(g * HPC, (g + 1) * HPC)
        eb = np.exp(rel_pos_bias[hs].astype(np.float32)).transpose(0, 2, 1)
        eb = eb * mask[b, 0].T[None].astype(np.float32)
        in_maps.append({
            "xqT": np.ascontiguousarray(query[b].T).astype(NP_F16),
            "xkvT": np.ascontiguousarray(key_value[b].T).astype(NP_F16),
            "wq": w_f16["Wq"][:, cs].copy(),
            "wk": w_f16["Wkv"][:, cs].copy(),
            "wv": w_f16["Wkv"][:, D_MODEL + cs.start : D_MODEL + cs.stop].copy(),
            "wo": w_f16["Wo"][cs, :].copy(),
            "expb": eb.astype(NP_F16),
        })
    return in_maps


def kernel(query, key_value, mask, rel_pos_bias, Wq, Wkv, Wo):
    global LAST_RESULTS
    query, key_value, mask, rel_pos_bias, Wq, Wkv, Wo = (
        np.asarray(a) for a in (query, key_value, mask, rel_pos_bias, Wq, Wkv, Wo)
    )
    nc = _get_nc()
    in_maps = _shard_inputs(query, key_value, mask, rel_pos_bias, Wq, Wkv, Wo)
    res = run_bass_kernel_spmd(nc, in_maps, core_ids=list(range(N_CORES)))
    LAST_RESULTS = res
    gpc = N_CORES // B  # cores per batch group
    out = np.stack([
        sum(res.results[b * gpc + i]["out"].astype(np.float32) for i in range(gpc))
        for b in range(B)
    ])
    return out
